# revision 29
# baseline (speedup 1.0000x reference)
"""Dense language-guidance cross-attention kernel for 8 Trainium2 cores.

Math (per batch b):
    K_v = vis @ W_vk.T + b_vk            (S, C)
    K_l = lang @ W_lk.T + b_lk           (N, C)
    V_v = vis @ W_vv.T + b_vv            (S, C)
    V_l = lang @ W_lv.T + b_lv           (N, C)
    A   = softmax_n(K_v @ K_l.T / sqrt(C))   (S, N)
    out = A @ V_l + A @ (A.T @ V_v)      (S, C)

Sharding: data-parallel over B — core i computes batch i end-to-end.

Device-side restructure (per core):
  * 1/sqrt(C) is folded into W_vk/b_vk on the host (exact: C**-0.5 == 2**-5).
  * softmax without max-subtraction (logits are ~N(0, 0.34); exp can't
    overflow), using unnormalized E = exp(logits):
        Z[s]  = sum_n E[s, n]
        out   = (E @ (V_l + X)) / Z[s]       where
        X     = (E/Z).T @ V_v                (N, C)
  * K_v is produced directly in transposed [c', s] layout (weights as the
    stationary operand) so the logits matmul has s (512) on the free dim.
  * logits live in [n, s] layout; PE-transpose gives the [s, n] copy needed
    for the X contraction, with Z computed by the transpose-copyout's
    accum_out for free.
  * all biases are fused into PSUM->SBUF copyouts (per-partition scalars for
    the transposed layouts, host-prebroadcast [128, C] tiles otherwise).
  * matmuls run as float32r (full-rate fp32 PE mode for free dim >= 256).
"""

import numpy as np

B, S, N, C = 8, 4096, 77, 1024
P = 128
CT = C // P          # 8 tiles over the feature dim
SCHUNK = 512         # s-chunk processed per main-loop iteration
NCHUNKS = S // SCHUNK
SBLK = SCHUNK // P   # 128-row blocks per chunk
NCORES = 8

_prog_cache = {}


def _build_program():
    if "nc" in _prog_cache:
        return _prog_cache["nc"]

    import concourse.bacc as bacc
    import concourse.mybir as mybir
    import concourse.tile as tile

    fp32 = mybir.dt.float32
    f32r = mybir.dt.float32r
    EXP = mybir.ActivationFunctionType.Exp
    COPY = mybir.ActivationFunctionType.Copy

    def r32(ap):
        return ap.bitcast(f32r)

    nc = bacc.Bacc()

    visT = nc.declare_dram_parameter("visT", [C, S], f32r, isOutput=False)
    langT = nc.declare_dram_parameter("langT", [C, N], f32r, isOutput=False)
    wvkT = nc.declare_dram_parameter("wvkT", [C, C], f32r, isOutput=False)
    wlkT = nc.declare_dram_parameter("wlkT", [C, C], f32r, isOutput=False)
    wvvT = nc.declare_dram_parameter("wvvT", [C, C], f32r, isOutput=False)
    wlvT = nc.declare_dram_parameter("wlvT", [C, C], f32r, isOutput=False)
    bvk_t = nc.declare_dram_parameter("bvk_t", [P, CT], fp32, isOutput=False)
    blk_t = nc.declare_dram_parameter("blk_t", [P, CT], fp32, isOutput=False)
    bvv_b = nc.declare_dram_parameter("bvv_b", [P, C], fp32, isOutput=False)
    blv_b = nc.declare_dram_parameter("blv_b", [P, C], fp32, isOutput=False)
    eye_d = nc.declare_dram_parameter("eye", [P, P], f32r, isOutput=False)
    out_d = nc.declare_dram_parameter("out", [S, C], fp32, isOutput=True)

    # [c, x] -> [p, ct, x] with c = ct*128 + p
    visT_r = visT.rearrange("(t p) s -> p t s", p=P)
    langT_r = langT.rearrange("(t p) n -> p t n", p=P)
    wvkT_r = wvkT.rearrange("(t p) n -> p t n", p=P)
    wlkT_r = wlkT.rearrange("(t p) n -> p t n", p=P)
    wvvT_r = wvvT.rearrange("(t p) n -> p t n", p=P)
    wlvT_r = wlvT.rearrange("(t p) n -> p t n", p=P)

    with tile.TileContext(nc) as tc, \
         tc.tile_pool(name="wbig", bufs=1) as wbig, \
         tc.tile_pool(name="wstream", bufs=2) as wstream, \
         tc.tile_pool(name="io", bufs=2) as io, \
         tc.tile_pool(name="persist", bufs=1) as persist, \
         tc.tile_pool(name="expat", bufs=NCHUNKS) as expat_pool, \
         tc.tile_pool(name="kvpool", bufs=2) as kvpool, \
         tc.tile_pool(name="vvpool", bufs=1) as vvpool, \
         tc.tile_pool(name="work", bufs=3) as work, \
         tc.tile_pool(name="psA", bufs=3, space="PSUM") as psA, \
         tc.tile_pool(name="psB", bufs=3, space="PSUM") as psB, \
         tc.tile_pool(name="psT", bufs=2, space="PSUM") as psT:

        bf16 = mybir.dt.bfloat16

        def absorb(ap):
            """Standalone LDWEIGHTS that takes over a freshly-DMA'd tile's
            sem wait on the PE.

            f32r matmuls lower to LDWEIGHTS+MATMUL whose LW slot carries at
            most ONE sync wait; this op observes the new DMA-queue semaphore
            first so the real matmuls after it never carry two. bf16 view
            because bass refuses standalone 4-byte ldweights; the loaded
            garbage weights are never used (every real matmul self-loads).
            """
            nc.tensor.ldweights(ap.bitcast(bf16)[:, :64])

        # ---- constants / small inputs --------------------------------
        eye = persist.tile([P, P], f32r)
        nc.sync.dma_start(out=eye[:], in_=eye_d[:])
        bvk = persist.tile([P, CT], fp32)
        nc.sync.dma_start(out=bvk[:], in_=bvk_t[:])
        blk = persist.tile([P, CT], fp32)
        nc.sync.dma_start(out=blk[:], in_=blk_t[:])
        bvv = persist.tile([P, C], fp32)
        nc.sync.dma_start(out=bvv[:], in_=bvv_b[:])
        blv = persist.tile([P, C], fp32)
        nc.sync.dma_start(out=blv[:], in_=blv_b[:])
        lT = persist.tile([P, CT, P], f32r)
        nc.vector.memset(lT[:].bitcast(fp32), 0.0)
        nc.sync.dma_start(out=lT[:, :, :N], in_=langT_r[:])

        absorb(eye[:, :])

        # ---- big resident weights (main loop) ------------------------
        wvk = wbig.tile([P, CT, C], f32r)
        nc.sync.dma_start(out=wvk[:], in_=wvkT_r[:])
        wvv = wbig.tile([P, CT, C], f32r)
        nc.sync.dma_start(out=wvv[:], in_=wvvT_r[:])
        absorb(wvv[:, 0, :])

        # ---- prologue: language projections --------------------------
        # K_l natural [n, c'] (no bias yet), V_l natural [n, c'] (+b_lv).
        kl = persist.tile([P, C], f32r)   # rows 0..76 valid, rest zero
        nc.vector.memset(kl[:].bitcast(fp32), 0.0)
        vl = persist.tile([P, C], fp32)
        for dst, w_r, bias in ((kl, wlkT_r, None), (vl, wlvT_r, blv)):
            for cc in range(2):
                ps = psB.tile([P, 512], fp32, name="ps_prolog", tag="acc512")
                wt = wstream.tile([P, CT, 512], f32r, name="wl_slab")
                for k in range(CT):
                    nc.sync.dma_start(out=wt[:, k, :],
                                      in_=w_r[:, k, cc * 512:(cc + 1) * 512])
                for k in range(CT):
                    nc.tensor.matmul(
                        ps[:, :], r32(lT[:, k, :]), r32(wt[:, k, :]),
                        start=(k == 0), stop=(k == CT - 1),
                    )
                sl = slice(cc * 512, (cc + 1) * 512)
                if bias is None:
                    nc.vector.tensor_copy(dst[:N, sl], ps[:N, :])
                else:
                    nc.vector.tensor_add(dst[:N, sl], ps[:N, :], bias[:N, sl])

        # K_l -> K_lT [c', n] via PE transpose, +b_lk on copyout.
        klT = persist.tile([P, CT, P], f32r)
        nc.vector.memset(klT[:].bitcast(fp32), 0.0)
        for t in range(CT):
            pst = psT.tile([P, P], f32r, name="pst_kl", tag="tp")
            nc.tensor.transpose(
                pst[:, :], kl[:, t * P:(t + 1) * P], eye[:, :]
            )
            nc.vector.tensor_tensor(
                klT[:, t, :N], pst[:, :N],
                blk[:, t:t + 1].to_broadcast([P, N]), mybir.AluOpType.add)

        # ---- persistent accumulators ---------------------------------
        x_acc = persist.tile([P, C], fp32)     # X = (E/Z).T @ V_v, rows 0..76
        nc.vector.memset(x_acc[:N, :], 0.0)
        rz_all = persist.tile([P, S // P], fp32)   # 1/Z, [s mod 128, s // 128]

        expat_tiles = []

        # ================= pass 1: over s-chunks ======================
        for ch in range(NCHUNKS):
            s0 = ch * SCHUNK
            vt = io.tile([P, CT, SCHUNK], f32r, name="vis_chunk")
            for k in range(CT):
                nc.sync.dma_start(out=vt[:, k, :],
                                  in_=visT_r[:, k, s0:s0 + SCHUNK])

            ea = expat_pool.tile([P, SCHUNK], f32r, name="expat")
            nc.vector.memset(ea[64:, :].bitcast(fp32), 0.0)  # rows 64..76 overwritten by exp below
            lg = psB.tile([P, SCHUNK], fp32, name="ps_logits", tag="acc512")

            # K_v^T tiles + logits accumulation (logits[n, s] = K_l @ K_v^T)
            for t in range(CT):
                kps = psA.tile([P, SCHUNK], fp32, name="ps_kv", tag="mm512")
                for k in range(CT):
                    nc.tensor.matmul(
                        kps[:], r32(wvk[:, k, t * P:(t + 1) * P]), r32(vt[:, k, :]),
                        start=(k == 0), stop=(k == CT - 1),
                    )
                kv = kvpool.tile([P, SCHUNK], f32r, name="kv_tile")
                nc.vector.tensor_tensor(
                    kv[:], kps[:],
                    bvk[:, t:t + 1].to_broadcast([P, SCHUNK]),
                    mybir.AluOpType.add)
                nc.tensor.matmul(
                    lg[:, :], r32(klT[:, t, :]), r32(kv[:]),
                    start=(t == 0), stop=(t == CT - 1),
                    skip_group_check=True,
                )

            # V_v for this chunk: [s, c'], bias fused on copyout
            vv = vvpool.tile([P, SBLK, C], f32r, name="vv_tile")
            for b in range(SBLK):
                for cc in range(2):
                    vps = psA.tile([P, SCHUNK], fp32, name="ps_vv", tag="mm512")
                    for k in range(CT):
                        nc.tensor.matmul(
                            vps[:], r32(vt[:, k, b * P:(b + 1) * P]),
                            r32(wvv[:, k, cc * 512:(cc + 1) * 512]),
                            start=(k == 0), stop=(k == CT - 1),
                        )
                    nc.vector.tensor_add(
                        vv[:, b, cc * 512:(cc + 1) * 512], vps[:],
                        bvv[:, cc * 512:(cc + 1) * 512],
                    )

            # E = exp(logits) in [n, s] layout (kept resident for pass 2)
            nc.scalar.activation(ea[:N, :], lg[:N, :], EXP)

            # per 128-row block: transpose -> [s, n], Z, 1/Z, E/Z, X matmuls
            atil = []
            for b in range(SBLK):
                pst = psT.tile([P, P], f32r, name="pst_a", tag="tp")
                nc.tensor.transpose(
                    pst[:, :], ea[:, b * P:(b + 1) * P], eye[:, :]
                )
                easb = work.tile([P, N], fp32, name="easb")
                zcol = work.tile([P, 1], fp32, name="zcol")
                nc.vector.tensor_copy(easb[:], pst[:, :N])
                nc.vector.reduce_sum(zcol[:], easb[:], axis=mybir.AxisListType.X)
                rzc = rz_all[:, ch * SBLK + b: ch * SBLK + b + 1]
                nc.vector.reciprocal(rzc, zcol[:])
                an = work.tile([P, P], f32r, name="a_norm")
                nc.vector.memset(an[:, N:].bitcast(fp32), 0.0)
                nc.vector.tensor_tensor(
                    an[:, :N], easb[:], rzc.to_broadcast([P, N]),
                    mybir.AluOpType.mult)
                atil.append(an)
            for cc in range(2):
                xps = psB.tile([P, SCHUNK], fp32, name="ps_x", tag="acc512")
                for b in range(SBLK):
                    nc.tensor.matmul(
                        xps[:, :], r32(atil[b][:]),
                        r32(vv[:, b, cc * 512:(cc + 1) * 512]),
                        start=(b == 0), stop=(b == SBLK - 1),
                        skip_group_check=True,
                    )
                nc.vector.tensor_add(
                    x_acc[:N, cc * 512:(cc + 1) * 512],
                    x_acc[:N, cc * 512:(cc + 1) * 512], xps[:N, :],
                )

            expat_tiles.append(ea)

        # ================= pass 2: out = (E @ (V_l + X)) / Z ==========
        wx = persist.tile([P, C], f32r)
        nc.vector.memset(wx[:].bitcast(fp32), 0.0)
        nc.vector.tensor_add(wx[:N, :], vl[:N, :], x_acc[:N, :])

        for ch in range(NCHUNKS):
            ea = expat_tiles[ch]
            for b in range(SBLK):
                rzc = rz_all[:, ch * SBLK + b: ch * SBLK + b + 1]
                r0 = ch * SCHUNK + b * P
                for cc in range(2):
                    ops_ = psA.tile([P, SCHUNK], fp32, name="ps_out", tag="mm512")
                    nc.tensor.matmul(
                        ops_[:, :], r32(ea[:, b * P:(b + 1) * P]),
                        r32(wx[:, cc * 512:(cc + 1) * 512]),
                        start=True, stop=True,
                    )
                    mid = work.tile([P, SCHUNK], fp32, name="mid_out", bufs=3)
                    nc.vector.tensor_tensor(
                        mid[:], ops_[:, :], rzc.to_broadcast([P, SCHUNK]),
                        mybir.AluOpType.mult)
                    nc.sync.dma_start(
                        out=out_d[r0:r0 + P, cc * 512:(cc + 1) * 512], in_=mid[:])

    nc.compile()
    _prog_cache["nc"] = nc
    return nc


def _make_in_maps(inputs):
    vis_features = inputs["vis_features"]
    lang_features = inputs["lang_features"]
    W_vk, b_vk = inputs["W_vk"], inputs["b_vk"]
    W_lk, b_lk = inputs["W_lk"], inputs["b_lk"]
    W_vv, b_vv = inputs["W_vv"], inputs["b_vv"]
    W_lv, b_lv = inputs["W_lv"], inputs["b_lv"]
    assert vis_features.shape == (B, S, C) and lang_features.shape == (B, N, C)

    f = np.float32
    scale = f(C) ** f(-0.5)  # 2**-5, exact
    wvkT = np.ascontiguousarray((W_vk * scale).T.astype(f))
    wlkT = np.ascontiguousarray(W_lk.T.astype(f))
    wvvT = np.ascontiguousarray(W_vv.T.astype(f))
    wlvT = np.ascontiguousarray(W_lv.T.astype(f))
    bvk_t = np.ascontiguousarray((b_vk * scale).astype(f).reshape(CT, P).T)
    blk_t = np.ascontiguousarray(b_lk.astype(f).reshape(CT, P).T)
    bvv_b = np.ascontiguousarray(np.broadcast_to(b_vv.astype(f), (P, C)))
    blv_b = np.ascontiguousarray(np.broadcast_to(b_lv.astype(f), (P, C)))
    eye = np.eye(P, dtype=f)

    shared = dict(wvkT=wvkT, wlkT=wlkT, wvvT=wvvT, wlvT=wlvT, bvk_t=bvk_t,
                  blk_t=blk_t, bvv_b=bvv_b, blv_b=blv_b, eye=eye)
    in_maps = []
    for b in range(B):
        m = dict(shared)
        m["visT"] = np.ascontiguousarray(vis_features[b].T.astype(f))
        m["langT"] = np.ascontiguousarray(lang_features[b].T.astype(f))
        in_maps.append(m)
    return in_maps


def kernel(**inputs):
    in_maps = _make_in_maps(inputs)
    nc = _build_program()
    from concourse.bass_utils import run_bass_kernel_spmd
    res = run_bass_kernel_spmd(nc, in_maps, list(range(NCORES)))
    return np.stack([res.results[i]["out"] for i in range(NCORES)], axis=0)


# revision 30
# speedup vs baseline: 1.0299x; 1.0299x over previous
"""Dense language-guidance cross-attention kernel for 8 Trainium2 cores.

Math (per batch b):
    K_v = vis @ W_vk.T + b_vk            (S, C)
    K_l = lang @ W_lk.T + b_lk           (N, C)
    V_v = vis @ W_vv.T + b_vv            (S, C)
    V_l = lang @ W_lv.T + b_lv           (N, C)
    A   = softmax_n(K_v @ K_l.T / sqrt(C))   (S, N)
    out = A @ V_l + A @ (A.T @ V_v)      (S, C)

Sharding: data-parallel over B — core i computes batch i end-to-end.

Device-side restructure (per core):
  * 1/sqrt(C) is folded into W_vk/b_vk on the host (exact: C**-0.5 == 2**-5).
  * softmax without max-subtraction (logits are ~N(0, 0.34); exp can't
    overflow), using unnormalized E = exp(logits):
        Z[s]  = sum_n E[s, n]
        out   = (E @ (V_l + X)) / Z[s]       where
        X     = (E/Z).T @ V_v                (N, C)
  * K_v is produced directly in transposed [c', s] layout (weights as the
    stationary operand) so the logits matmul has s (512) on the free dim.
  * logits live in [n, s] layout; PE-transpose gives the [s, n] copy needed
    for the X contraction, with Z computed by the transpose-copyout's
    accum_out for free.
  * all biases are fused into PSUM->SBUF copyouts (per-partition scalars for
    the transposed layouts, host-prebroadcast [128, C] tiles otherwise).
  * matmuls run as float32r (full-rate fp32 PE mode for free dim >= 256).
"""

import numpy as np

B, S, N, C = 8, 4096, 77, 1024
P = 128
CT = C // P          # 8 tiles over the feature dim
SCHUNK = 512         # s-chunk processed per main-loop iteration
NCHUNKS = S // SCHUNK
SBLK = SCHUNK // P   # 128-row blocks per chunk
NCORES = 8

_prog_cache = {}


def _build_program():
    if "nc" in _prog_cache:
        return _prog_cache["nc"]

    import concourse.bacc as bacc
    import concourse.mybir as mybir
    import concourse.tile as tile

    fp32 = mybir.dt.float32
    f32r = mybir.dt.float32r
    EXP = mybir.ActivationFunctionType.Exp
    COPY = mybir.ActivationFunctionType.Copy

    def r32(ap):
        return ap.bitcast(f32r)

    nc = bacc.Bacc()

    visT = nc.declare_dram_parameter("visT", [C, S], f32r, isOutput=False)
    langT = nc.declare_dram_parameter("langT", [C, N], f32r, isOutput=False)
    wvkT = nc.declare_dram_parameter("wvkT", [C, C], f32r, isOutput=False)
    wlkT = nc.declare_dram_parameter("wlkT", [C, C], f32r, isOutput=False)
    wvvT = nc.declare_dram_parameter("wvvT", [C, C], f32r, isOutput=False)
    wlvT = nc.declare_dram_parameter("wlvT", [C, C], f32r, isOutput=False)
    bvk_t = nc.declare_dram_parameter("bvk_t", [P, CT], fp32, isOutput=False)
    blk_t = nc.declare_dram_parameter("blk_t", [P, CT], fp32, isOutput=False)
    bvv_b = nc.declare_dram_parameter("bvv_b", [P, C], fp32, isOutput=False)
    blv_b = nc.declare_dram_parameter("blv_b", [P, C], fp32, isOutput=False)
    eye_d = nc.declare_dram_parameter("eye", [P, P], f32r, isOutput=False)
    out_d = nc.declare_dram_parameter("out", [S, C], fp32, isOutput=True)

    # [c, x] -> [p, ct, x] with c = ct*128 + p
    visT_r = visT.rearrange("(t p) s -> p t s", p=P)
    langT_r = langT.rearrange("(t p) n -> p t n", p=P)
    wvkT_r = wvkT.rearrange("(t p) n -> p t n", p=P)
    wlkT_r = wlkT.rearrange("(t p) n -> p t n", p=P)
    wvvT_r = wvvT.rearrange("(t p) n -> p t n", p=P)
    wlvT_r = wlvT.rearrange("(t p) n -> p t n", p=P)

    with tile.TileContext(nc) as tc, \
         tc.tile_pool(name="wbig", bufs=1) as wbig, \
         tc.tile_pool(name="wstream", bufs=2) as wstream, \
         tc.tile_pool(name="io", bufs=2) as io, \
         tc.tile_pool(name="persist", bufs=1) as persist, \
         tc.tile_pool(name="expat", bufs=NCHUNKS) as expat_pool, \
         tc.tile_pool(name="kvpool", bufs=2) as kvpool, \
         tc.tile_pool(name="vvpool", bufs=1) as vvpool, \
         tc.tile_pool(name="work", bufs=3) as work, \
         tc.tile_pool(name="psA", bufs=3, space="PSUM") as psA, \
         tc.tile_pool(name="psB", bufs=3, space="PSUM") as psB, \
         tc.tile_pool(name="psT", bufs=2, space="PSUM") as psT:

        bf16 = mybir.dt.bfloat16

        def absorb(ap):
            """Standalone LDWEIGHTS that takes over a freshly-DMA'd tile's
            sem wait on the PE.

            f32r matmuls lower to LDWEIGHTS+MATMUL whose LW slot carries at
            most ONE sync wait; this op observes the new DMA-queue semaphore
            first so the real matmuls after it never carry two. bf16 view
            because bass refuses standalone 4-byte ldweights; the loaded
            garbage weights are never used (every real matmul self-loads).
            """
            nc.tensor.ldweights(ap.bitcast(bf16)[:, :64])

        # ---- constants / small inputs --------------------------------
        eye = persist.tile([P, P], f32r)
        nc.sync.dma_start(out=eye[:], in_=eye_d[:])
        bvk = persist.tile([P, CT], fp32)
        nc.sync.dma_start(out=bvk[:], in_=bvk_t[:])
        blk = persist.tile([P, CT], fp32)
        nc.sync.dma_start(out=blk[:], in_=blk_t[:])
        bvv = persist.tile([P, C], fp32)
        nc.sync.dma_start(out=bvv[:], in_=bvv_b[:])
        blv = persist.tile([P, C], fp32)
        nc.sync.dma_start(out=blv[:], in_=blv_b[:])
        lT = persist.tile([P, CT, P], f32r)
        nc.vector.memset(lT[:].bitcast(fp32), 0.0)
        nc.sync.dma_start(out=lT[:, :, :N], in_=langT_r[:])

        absorb(lT[:, 0, :])
        absorb(eye[:, :])
        # DVE touches: absorb the bias tiles' DMA-queue waits onto the DVE
        # proc so bias-fused copyouts never carry a second (external) wait.
        dve_touch = persist.tile([P, 4], fp32)
        nc.vector.tensor_copy(dve_touch[:, 0:1], bvk[:, 0:1])
        nc.vector.tensor_copy(dve_touch[:, 1:2], blk[:, 0:1])
        nc.vector.tensor_copy(dve_touch[:, 2:3], bvv[:, 0:1])
        nc.vector.tensor_copy(dve_touch[:, 3:4], blv[:, 0:1])

        # ---- big resident weights (main loop) ------------------------
        wvk = wbig.tile([P, CT, C], f32r)
        nc.sync.dma_start(out=wvk[:], in_=wvkT_r[:])
        wvv = wbig.tile([P, CT, C], f32r)
        nc.sync.dma_start(out=wvv[:], in_=wvvT_r[:])
        absorb(wvk[:, 0, :])
        absorb(wvv[:, 0, :])

        # ---- prologue: language projections --------------------------
        # K_l natural [n, c'] (no bias yet), V_l natural [n, c'] (+b_lv).
        kl = persist.tile([P, C], f32r)   # rows 0..76 valid, rest zero
        nc.vector.memset(kl[:].bitcast(fp32), 0.0)
        vl = persist.tile([P, C], fp32)
        for dst, w_r, bias in ((kl, wlkT_r, None), (vl, wlvT_r, blv)):
            for cc in range(2):
                ps = psB.tile([P, 512], fp32, name="ps_prolog", tag="acc512")
                wt = wstream.tile([P, CT, 512], f32r, name="wl_slab")
                for k in range(CT):
                    nc.sync.dma_start(out=wt[:, k, :],
                                      in_=w_r[:, k, cc * 512:(cc + 1) * 512])
                absorb(wt[:, 0, :])
                for k in range(CT):
                    nc.tensor.matmul(
                        ps[:, :], r32(lT[:, k, :]), r32(wt[:, k, :]),
                        start=(k == 0), stop=(k == CT - 1),
                    )
                sl = slice(cc * 512, (cc + 1) * 512)
                if bias is None:
                    nc.vector.tensor_copy(dst[:N, sl], ps[:N, :])
                else:
                    nc.vector.tensor_add(dst[:N, sl], ps[:N, :], bias[:N, sl])

        # K_l -> K_lT [c', n] via PE transpose, +b_lk on copyout.
        klT = persist.tile([P, CT, P], f32r)
        nc.vector.memset(klT[:].bitcast(fp32), 0.0)
        for t in range(CT):
            pst = psT.tile([P, P], f32r, name="pst_kl", tag="tp")
            nc.tensor.transpose(
                pst[:, :], kl[:, t * P:(t + 1) * P], eye[:, :]
            )
            nc.vector.tensor_tensor(
                klT[:, t, :N], pst[:, :N],
                blk[:, t:t + 1].to_broadcast([P, N]), mybir.AluOpType.add)

        # ---- persistent accumulators ---------------------------------
        x_acc = persist.tile([P, C], fp32)     # X = (E/Z).T @ V_v, rows 0..76
        nc.vector.memset(x_acc[:N, :], 0.0)
        rz_all = persist.tile([P, S // P], fp32)   # 1/Z, [s mod 128, s // 128]

        expat_tiles = []

        # ================= pass 1: over s-chunks ======================
        for ch in range(NCHUNKS):
            s0 = ch * SCHUNK
            vt = io.tile([P, CT, SCHUNK], f32r, name="vis_chunk")
            for k in range(CT):
                nc.sync.dma_start(out=vt[:, k, :],
                                  in_=visT_r[:, k, s0:s0 + SCHUNK])
            absorb(vt[:, 0, :])

            ea = expat_pool.tile([P, SCHUNK], f32r, name="expat")
            nc.vector.memset(ea[64:, :].bitcast(fp32), 0.0)  # rows 64..76 overwritten by exp below
            lg = psB.tile([P, SCHUNK], fp32, name="ps_logits", tag="acc512")

            # K_v^T tiles + logits accumulation (logits[n, s] = K_l @ K_v^T)
            for t in range(CT):
                kps = psA.tile([P, SCHUNK], fp32, name="ps_kv", tag="mm512")
                for k in range(CT):
                    nc.tensor.matmul(
                        kps[:], r32(wvk[:, k, t * P:(t + 1) * P]), r32(vt[:, k, :]),
                        start=(k == 0), stop=(k == CT - 1),
                    )
                kv = kvpool.tile([P, SCHUNK], f32r, name="kv_tile")
                nc.vector.tensor_tensor(
                    kv[:], kps[:],
                    bvk[:, t:t + 1].to_broadcast([P, SCHUNK]),
                    mybir.AluOpType.add)
                nc.tensor.matmul(
                    lg[:, :], r32(klT[:, t, :]), r32(kv[:]),
                    start=(t == 0), stop=(t == CT - 1),
                    skip_group_check=True,
                )

            # V_v for this chunk: [s, c'], bias fused on copyout
            vv = vvpool.tile([P, SBLK, C], f32r, name="vv_tile")
            for b in range(SBLK):
                for cc in range(2):
                    vps = psA.tile([P, SCHUNK], fp32, name="ps_vv", tag="mm512")
                    for k in range(CT):
                        nc.tensor.matmul(
                            vps[:], r32(vt[:, k, b * P:(b + 1) * P]),
                            r32(wvv[:, k, cc * 512:(cc + 1) * 512]),
                            start=(k == 0), stop=(k == CT - 1),
                        )
                    nc.vector.tensor_add(
                        vv[:, b, cc * 512:(cc + 1) * 512], vps[:],
                        bvv[:, cc * 512:(cc + 1) * 512],
                    )

            # E = exp(logits) in [n, s] layout (kept resident for pass 2)
            nc.scalar.activation(ea[:N, :], lg[:N, :], EXP)

            # per 128-row block: transpose -> [s, n], Z, 1/Z, E/Z, X matmuls
            atil = []
            for b in range(SBLK):
                pst = psT.tile([P, P], f32r, name="pst_a", tag="tp")
                nc.tensor.transpose(
                    pst[:, :], ea[:, b * P:(b + 1) * P], eye[:, :]
                )
                easb = work.tile([P, N], fp32, name="easb")
                zcol = work.tile([P, 1], fp32, name="zcol")
                nc.vector.tensor_copy(easb[:], pst[:, :N])
                nc.vector.reduce_sum(zcol[:], easb[:], axis=mybir.AxisListType.X)
                rzc = rz_all[:, ch * SBLK + b: ch * SBLK + b + 1]
                nc.vector.reciprocal(rzc, zcol[:])
                an = work.tile([P, P], f32r, name="a_norm")
                nc.vector.memset(an[:, N:].bitcast(fp32), 0.0)
                nc.vector.tensor_tensor(
                    an[:, :N], easb[:], rzc.to_broadcast([P, N]),
                    mybir.AluOpType.mult)
                atil.append(an)
            for cc in range(2):
                xps = psB.tile([P, SCHUNK], fp32, name="ps_x", tag="acc512")
                for b in range(SBLK):
                    nc.tensor.matmul(
                        xps[:, :], r32(atil[b][:]),
                        r32(vv[:, b, cc * 512:(cc + 1) * 512]),
                        start=(b == 0), stop=(b == SBLK - 1),
                        skip_group_check=True,
                    )
                nc.vector.tensor_add(
                    x_acc[:N, cc * 512:(cc + 1) * 512],
                    x_acc[:N, cc * 512:(cc + 1) * 512], xps[:N, :],
                )

            expat_tiles.append(ea)

        # ================= pass 2: out = (E @ (V_l + X)) / Z ==========
        wx = persist.tile([P, C], f32r)
        nc.vector.memset(wx[:].bitcast(fp32), 0.0)
        nc.vector.tensor_add(wx[:N, :], vl[:N, :], x_acc[:N, :])

        for ch in range(NCHUNKS):
            ea = expat_tiles[ch]
            for b in range(SBLK):
                rzc = rz_all[:, ch * SBLK + b: ch * SBLK + b + 1]
                r0 = ch * SCHUNK + b * P
                for cc in range(2):
                    ops_ = psA.tile([P, SCHUNK], fp32, name="ps_out", tag="mm512")
                    nc.tensor.matmul(
                        ops_[:, :], r32(ea[:, b * P:(b + 1) * P]),
                        r32(wx[:, cc * 512:(cc + 1) * 512]),
                        start=True, stop=True,
                    )
                    mid = work.tile([P, SCHUNK], fp32, name="mid_out", bufs=3)
                    nc.vector.tensor_tensor(
                        mid[:], ops_[:, :], rzc.to_broadcast([P, SCHUNK]),
                        mybir.AluOpType.mult)
                    nc.sync.dma_start(
                        out=out_d[r0:r0 + P, cc * 512:(cc + 1) * 512], in_=mid[:])

    nc.compile()
    _prog_cache["nc"] = nc
    return nc


def _make_in_maps(inputs):
    vis_features = inputs["vis_features"]
    lang_features = inputs["lang_features"]
    W_vk, b_vk = inputs["W_vk"], inputs["b_vk"]
    W_lk, b_lk = inputs["W_lk"], inputs["b_lk"]
    W_vv, b_vv = inputs["W_vv"], inputs["b_vv"]
    W_lv, b_lv = inputs["W_lv"], inputs["b_lv"]
    assert vis_features.shape == (B, S, C) and lang_features.shape == (B, N, C)

    f = np.float32
    scale = f(C) ** f(-0.5)  # 2**-5, exact
    wvkT = np.ascontiguousarray((W_vk * scale).T.astype(f))
    wlkT = np.ascontiguousarray(W_lk.T.astype(f))
    wvvT = np.ascontiguousarray(W_vv.T.astype(f))
    wlvT = np.ascontiguousarray(W_lv.T.astype(f))
    bvk_t = np.ascontiguousarray((b_vk * scale).astype(f).reshape(CT, P).T)
    blk_t = np.ascontiguousarray(b_lk.astype(f).reshape(CT, P).T)
    bvv_b = np.ascontiguousarray(np.broadcast_to(b_vv.astype(f), (P, C)))
    blv_b = np.ascontiguousarray(np.broadcast_to(b_lv.astype(f), (P, C)))
    eye = np.eye(P, dtype=f)

    shared = dict(wvkT=wvkT, wlkT=wlkT, wvvT=wvvT, wlvT=wlvT, bvk_t=bvk_t,
                  blk_t=blk_t, bvv_b=bvv_b, blv_b=blv_b, eye=eye)
    in_maps = []
    for b in range(B):
        m = dict(shared)
        m["visT"] = np.ascontiguousarray(vis_features[b].T.astype(f))
        m["langT"] = np.ascontiguousarray(lang_features[b].T.astype(f))
        in_maps.append(m)
    return in_maps


def kernel(**inputs):
    in_maps = _make_in_maps(inputs)
    nc = _build_program()
    from concourse.bass_utils import run_bass_kernel_spmd
    res = run_bass_kernel_spmd(nc, in_maps, list(range(NCORES)))
    return np.stack([res.results[i]["out"] for i in range(NCORES)], axis=0)
